# revision 8
# baseline (speedup 1.0000x reference)
"""Trainium2 Bass kernel for MultiHeadAttention with gene-regulatory bias.

Reference computation (per batch b):
    q,k,v = (x @ W + b) split into 8 heads of 64
    scores = q k^T / 8 + gene[None]        (gene shared across batch/heads)
    scores = where(mask==0, -inf, scores)
    P = softmax(scores, axis=-1)           -> returned as attention_weights
    out = (P v) concat-heads @ Wo + bo

Sharding: 8 cores = (batch b, query-half). Each core handles 1024 query rows
x full 2048 keys x all 8 heads. No collectives; host reassembles.

Device dataflow (per core) is "transposed-score" oriented: S^T[k,q] tiles are
computed natively on the PE (so the P@V contraction needs no on-chip transpose
of the big score tensor), the softmax denominator is obtained by appending a
ones-column to V (one extra PSUM row in the P@V matmul), and normalization
happens along the free axis on the vector engine. The [k,q] -> [q,k] layout
fix of the returned weights is done on the host (pure layout/unshard step).

Numerics: scores matmul in fp32; gene/mask bias, exp(S^T) tile and V in fp16
(~5e-4 rel err); exp is shifted by -4 (softmax-invariant) to stay inside
fp16 range; all accumulations in fp32 PSUM.
"""

import numpy as np
from contextlib import ExitStack

import concourse.bass as bass
import concourse.bacc as bacc
import concourse.tile as tile
from concourse import mybir
from concourse.masks import make_identity
from concourse.bass_utils import run_bass_kernel_spmd

F32 = mybir.dt.float32
F16 = mybir.dt.float16
I32 = mybir.dt.int32

B, L, E = 4, 2048, 512
H, DH = 8, 64
NCORES = 8
LQ = L // 2          # query rows per core
LK = L
NEG = -10000.0       # "minus infinity" for masked scores
ESHIFT = -4.0        # exp(s + ESHIFT): softmax-invariant fp16 range guard


def build_nc(lq=LQ, lk=LK):
    """Build the per-core Bass program (SPMD: same program, per-core data)."""
    nc = bacc.Bacc()
    qb = min(512, lq)            # query block for matmul moving operand
    nq = lq // qb                # query blocks
    k16 = lk // 128              # 128-row key chunks

    # ---- DRAM I/O ----
    qT_d = nc.dram_tensor("qT", [E, lq], F32, kind="ExternalInput")
    kT_d = nc.dram_tensor("kT", [E, lk], F32, kind="ExternalInput")
    vT_d = nc.dram_tensor("vT", [E, lk], F32, kind="ExternalInput")
    maskT_d = nc.dram_tensor("maskT", [lk, lq], I32, kind="ExternalInput")
    geneT_d = nc.dram_tensor("geneT", [lk, lq], F32, kind="ExternalInput")
    Wq_d = nc.dram_tensor("Wq", [E, E], F32, kind="ExternalInput")
    Wk_d = nc.dram_tensor("Wk", [E, E], F32, kind="ExternalInput")
    Wv_d = nc.dram_tensor("Wv", [E, E], F32, kind="ExternalInput")
    Wo_d = nc.dram_tensor("Wo", [E, E], F32, kind="ExternalInput")
    bq_d = nc.dram_tensor("bq", [E], F32, kind="ExternalInput")
    bk_d = nc.dram_tensor("bk", [E], F32, kind="ExternalInput")
    bv_d = nc.dram_tensor("bv", [E], F32, kind="ExternalInput")
    bo_d = nc.dram_tensor("bo", [E], F32, kind="ExternalInput")
    attnwT_d = nc.dram_tensor("attnwT", [H, lk, lq], F32, kind="ExternalOutput")
    outT_d = nc.dram_tensor("outT", [E, lq], F32, kind="ExternalOutput")

    # DRAM views with 128-partition layout
    att_r = attnwT_d.rearrange("h (n p) q -> h p n q", p=128)  # [H,128,k16,lq]

    with tile.TileContext(nc) as tc, ExitStack() as ctx:
        const = ctx.enter_context(tc.tile_pool(name="const", bufs=1))
        persist = ctx.enter_context(tc.tile_pool(name="persist", bufs=1))

        ident16 = const.tile([128, 128], F16, tag="id16")
        make_identity(nc, ident16)
        ones128 = const.tile([1, 128], F32, tag="ones")
        nc.vector.memset(ones128, 1.0)
        bv_row = const.tile([1, E], F32, tag="bvrow")
        nc.sync.dma_start(out=bv_row, in_=bv_d.rearrange("(o e) -> o e", o=1))
        bq_s = const.tile([128, 4], F32, tag="bq")
        nc.sync.dma_start(out=bq_s, in_=bq_d.rearrange("(c p) -> p c", p=128))
        nc.scalar.mul(out=bq_s, in_=bq_s, mul=0.125)  # fold 1/sqrt(dh) into q
        bk_s = const.tile([128, 4], F32, tag="bk")
        nc.sync.dma_start(out=bk_s, in_=bk_d.rearrange("(c p) -> p c", p=128))
        bo_s = const.tile([128, 4], F32, tag="bo")
        nc.sync.dma_start(out=bo_s, in_=bo_d.rearrange("(c p) -> p c", p=128))
        eshift = const.tile([128, 1], F32, tag="eshift")
        nc.vector.memset(eshift, ESHIFT)

        QT = persist.tile([128, 4, lq], F32, tag="QT")     # q^T/8+bq/8 [E,lq]
        KT = persist.tile([128, 4, lk], F32, tag="KT")     # k^T        [E,lk]
        V2 = persist.tile([128, k16, H, DH + 1], F16, tag="V2")  # [V_h | 1]
        SBT = persist.tile([128, k16, lq], F16, tag="SBT")  # masked gene^T
        OT = persist.tile([128, 4, lq], F32, tag="OT")     # attn out^T [E,lq]
        Wo_sb = persist.tile([128, 4, E], F32, tag="Wo")
        nc.sync.dma_start(out=Wo_sb, in_=Wo_d.rearrange("(c p) e -> p c e", p=128))

        nc.gpsimd.memset(V2[:, :, :, DH:DH + 1], 1.0)  # denominator ones-col

        # ---------- Phase 1: projections ----------
        with tc.tile_pool(name="ph1x", bufs=1) as xpool, \
             tc.tile_pool(name="ph1w", bufs=1) as wpool, \
             tc.tile_pool(name="ph1ps", bufs=4, space="PSUM") as pp:

            def load_xw(x_dram, w_dram, xlen):
                xT = xpool.tile([128, 4, xlen], F32, tag="x")
                nc.sync.dma_start(out=xT, in_=x_dram.rearrange("(c p) n -> p c n", p=128))
                W = wpool.tile([128, 4, E], F32, tag="w")
                nc.sync.dma_start(out=W, in_=w_dram.rearrange("(c p) e -> p c e", p=128))
                return xT, W

            # Q^T = Wq^T x^T  (scaled by 1/8, bias bq/8)
            xT, W = load_xw(qT_d, Wq_d, lq)
            for m in range(4):
                for qc in range(nq):
                    ps = pp.tile([128, qb], F32, tag="ps")
                    for c in range(4):
                        nc.tensor.matmul(ps, lhsT=W[:, c, m * 128:(m + 1) * 128],
                                         rhs=xT[:, c, qc * qb:(qc + 1) * qb],
                                         start=(c == 0), stop=(c == 3))
                    nc.scalar.activation(out=QT[:, m, qc * qb:(qc + 1) * qb], in_=ps,
                                         func=mybir.ActivationFunctionType.Identity,
                                         bias=bq_s[:, m:m + 1], scale=0.125)
            # K^T = Wk^T x^T
            xT, W = load_xw(kT_d, Wk_d, lk)
            for m in range(4):
                for qc in range(lk // 512):
                    ps = pp.tile([128, 512], F32, tag="ps")
                    for c in range(4):
                        nc.tensor.matmul(ps, lhsT=W[:, c, m * 128:(m + 1) * 128],
                                         rhs=xT[:, c, qc * 512:(qc + 1) * 512],
                                         start=(c == 0), stop=(c == 3))
                    nc.scalar.activation(out=KT[:, m, qc * 512:(qc + 1) * 512], in_=ps,
                                         func=mybir.ActivationFunctionType.Identity,
                                         bias=bk_s[:, m:m + 1], scale=1.0)
            # V = value Wv + bv  (natural [k, dh] layout, per-head + ones col)
            xT, W = load_xw(vT_d, Wv_d, lk)
            for kc in range(k16):
                ps = pp.tile([128, E], F32, tag="psv")
                for c in range(4):
                    nc.tensor.matmul(ps, lhsT=xT[:, c, kc * 128:(kc + 1) * 128],
                                     rhs=W[:, c, :], start=(c == 0), stop=False)
                nc.tensor.matmul(ps, lhsT=ones128, rhs=bv_row,
                                 start=False, stop=True)  # += bv broadcast
                nc.scalar.activation(
                    out=V2[:, kc, :, 0:DH],
                    in_=ps.rearrange("p (h d) -> p h d", h=H),
                    func=mybir.ActivationFunctionType.Copy)

        # ---------- Phase 2a: masked gene bias (transposed) ----------
        with tc.tile_pool(name="ph2a", bufs=2) as mg:
            for kc in range(k16):
                mt = mg.tile([128, lq], I32, tag="m")
                nc.sync.dma_start(out=mt, in_=maskT_d[kc * 128:(kc + 1) * 128, :])
                gt = mg.tile([128, lq], F32, tag="g")
                nc.sync.dma_start(out=gt, in_=geneT_d[kc * 128:(kc + 1) * 128, :])
                gf = mg.tile([128, lq], F16, tag="gf")
                nc.vector.tensor_copy(out=gf, in_=gt)     # f32 -> f16
                nc.gpsimd.memset(SBT[:, kc, :], NEG)
                nc.vector.copy_predicated(out=SBT[:, kc, :], mask=mt, data=gf)

        # ---------- Phase 2b: attention ----------
        with tc.tile_pool(name="spsum", bufs=4, space="PSUM") as spsum, \
             tc.tile_pool(name="opsum", bufs=2, space="PSUM") as opsum, \
             tc.tile_pool(name="rbpsum", bufs=1, space="PSUM") as rbpsum, \
             tc.tile_pool(name="strip", bufs=2) as strip, \
             tc.tile_pool(name="pout", bufs=3) as pout, \
             tc.tile_pool(name="small", bufs=4) as small:
            for h in range(H):
                hp, hc = (h % 2) * 64, h // 2
                for qg in range(nq):
                    qs = slice(qg * qb, (qg + 1) * qb)
                    Pt = strip.tile([128, k16, qb], F16, tag="pt")
                    for kc in range(k16):
                        ps = spsum.tile([128, qb], F32, tag="s")
                        # S^T[k,q] = K_h Q_h^T  (q pre-scaled by 1/8)
                        nc.tensor.matmul(ps,
                                         lhsT=KT[hp:hp + 64, hc, kc * 128:(kc + 1) * 128],
                                         rhs=QT[hp:hp + 64, hc, qs],
                                         start=True, stop=False)
                        # += gene/mask bias via identity weights
                        nc.tensor.matmul(ps, lhsT=ident16, rhs=SBT[:, kc, qs],
                                         start=False, stop=True)
                        nc.scalar.activation(out=Pt[:, kc, :], in_=ps,
                                             func=mybir.ActivationFunctionType.Exp,
                                             bias=eshift, scale=1.0)
                    # O'^T[dh,q] (+ denominator row) = [V_h|1]^T P~^T
                    ops = opsum.tile([DH + 1, qb], F32, tag="o")
                    for kc in range(k16):
                        nc.tensor.matmul(ops, lhsT=V2[:, kc, h, :], rhs=Pt[:, kc, :],
                                         start=(kc == 0), stop=(kc == k16 - 1))
                    rd = small.tile([1, qb], F32, tag="rd")
                    nc.vector.reciprocal(out=rd, in_=ops[DH:DH + 1, :])
                    rbp = rbpsum.tile([128, qb], F32, tag="rb")
                    nc.tensor.matmul(rbp, lhsT=ones128, rhs=rd, start=True, stop=True)
                    rb = small.tile([128, qb], F32, tag="rbs")
                    nc.vector.tensor_copy(out=rb, in_=rbp)
                    # normalized attention out^T rows for this head
                    nc.vector.tensor_mul(out=OT[hp:hp + 64, hc, qs],
                                         in0=ops[0:DH, :], in1=rb[0:DH, :])
                    # normalized weights -> DRAM, 4 key-chunks per DMA
                    for c4 in range(k16 // 4):
                        Pn = pout.tile([128, 4, qb], F32, tag="pn")
                        for j in range(4):
                            nc.vector.tensor_mul(out=Pn[:, j, :],
                                                 in0=Pt[:, c4 * 4 + j, :], in1=rb)
                        nc.sync.dma_start(
                            out=att_r[h, :, c4 * 4:(c4 + 1) * 4, qs], in_=Pn)

        # ---------- Phase 3: output projection ----------
        with tc.tile_pool(name="ph3ps", bufs=2, space="PSUM") as wp, \
             tc.tile_pool(name="ph3y", bufs=2) as yp:
            for qc in range(nq):
                for m in range(4):
                    ps = wp.tile([128, qb], F32, tag="y")
                    for c in range(4):
                        nc.tensor.matmul(ps, lhsT=Wo_sb[:, c, m * 128:(m + 1) * 128],
                                         rhs=OT[:, c, qc * qb:(qc + 1) * qb],
                                         start=(c == 0), stop=(c == 3))
                    y = yp.tile([128, qb], F32, tag="ysb")
                    nc.scalar.activation(out=y, in_=ps,
                                         func=mybir.ActivationFunctionType.Identity,
                                         bias=bo_s[:, m:m + 1], scale=1.0)
                    nc.sync.dma_start(out=outT_d[m * 128:(m + 1) * 128,
                                                 qc * qb:(qc + 1) * qb], in_=y)

    nc.finalize()
    return nc


_NC_CACHE = {}


def _get_nc():
    if "nc" not in _NC_CACHE:
        _NC_CACHE["nc"] = build_nc()
    return _NC_CACHE["nc"]


def make_in_maps(query, key, value, attn_mask, gene_regulatory_matrix,
                 Wq, bq, Wk, bk, Wv, bv, Wo, bo):
    f = lambda a: np.asarray(a, np.float32)
    query, key, value = f(query), f(key), f(value)
    gene = f(gene_regulatory_matrix)
    mask = np.asarray(attn_mask, np.int32)
    shared = {"Wq": f(Wq), "Wk": f(Wk), "Wv": f(Wv), "Wo": f(Wo),
              "bq": f(bq), "bk": f(bk), "bv": f(bv), "bo": f(bo)}
    in_maps = []
    for c in range(NCORES):
        b, half = c // 2, c % 2
        qs = slice(half * LQ, (half + 1) * LQ)
        in_maps.append({
            "qT": np.ascontiguousarray(query[b, qs].T),
            "kT": np.ascontiguousarray(key[b].T),
            "vT": np.ascontiguousarray(value[b].T),
            "maskT": np.ascontiguousarray(mask[b, qs].T),
            "geneT": np.ascontiguousarray(gene[qs].T),
            **shared,
        })
    return in_maps


def assemble(results):
    out = np.empty((B, L, E), np.float32)
    aw = np.empty((B, H, L, L), np.float32)
    for c in range(NCORES):
        b, half = c // 2, c % 2
        qs = slice(half * LQ, (half + 1) * LQ)
        aw[b, :, qs, :] = results[c]["attnwT"].transpose(0, 2, 1)
        out[b, qs, :] = results[c]["outT"].T
    return out, aw


def kernel(query, key, value, attn_mask, gene_regulatory_matrix,
           Wq, bq, Wk, bk, Wv, bv, Wo, bo):
    nc = _get_nc()
    in_maps = make_in_maps(query, key, value, attn_mask,
                           gene_regulatory_matrix,
                           Wq, bq, Wk, bk, Wv, bv, Wo, bo)
    res = run_bass_kernel_spmd(nc, in_maps, list(range(NCORES))).results
    return assemble(res)


# revision 13
# speedup vs baseline: 1.9146x; 1.9146x over previous
"""Trainium2 Bass kernel for MultiHeadAttention with gene-regulatory bias.

Reference computation (per batch b):
    q,k,v = (x @ W + b) split into 8 heads of 64
    scores = q k^T / 8 + gene[None]        (gene shared across batch/heads)
    scores = where(mask==0, -inf, scores)
    P = softmax(scores, axis=-1)           -> returned as attention_weights
    out = (P v) concat-heads @ Wo + bo

Sharding: 8 cores = (batch b, query-half). Each core handles 1024 query rows
x full 2048 keys x all 8 heads. No collectives; host reassembles.

Device dataflow (per core) is "transposed-score" oriented: S^T[k,q] tiles are
computed natively on the PE (so the P@V contraction needs no on-chip transpose
of the big score tensor), the softmax denominator is obtained by appending a
ones-column to V (one extra PSUM row in the P@V matmul), and normalization
happens along the free axis on the vector engine. The [k,q] -> [q,k] layout
fix of the returned weights is done on the host (pure layout/unshard step).

Numerics: fp32 matmuls run in the PE's slow LOW_HIGH mode, so every matmul is
16-bit. Precision-critical products (projections of q/k, q.k^T scores) use
3-term bf16 error-compensated splits (x*y ~= xh*yh + xh*yl + xl*yh, each term
bf16), giving ~1e-4 score accuracy. The gene/mask bias tile is fp16 (5e-4 of
gene). exp tiles, V and the output projection are plain bf16 (~4e-3 relative,
well under the absmax gate). exp is shifted by -4 (softmax-invariant range
guard); all accumulation is fp32 PSUM.
"""

import numpy as np
from contextlib import ExitStack

import concourse.bass as bass
import concourse.bacc as bacc
import concourse.tile as tile
from concourse import mybir
from concourse.masks import make_identity
from concourse.bass_utils import run_bass_kernel_spmd

F32 = mybir.dt.float32
F16 = mybir.dt.float16
BF16 = mybir.dt.bfloat16
I32 = mybir.dt.int32
AF = mybir.ActivationFunctionType

B, L, E = 4, 2048, 512
H, DH = 8, 64
NCORES = 8
LQ = L // 2          # query rows per core
LK = L
NEG = -10000.0       # "minus infinity" for masked scores
ESHIFT = -4.0        # exp(s + ESHIFT): softmax-invariant fp16/bf16 range guard


def build_nc(lq=LQ, lk=LK):
    """Build the per-core Bass program (SPMD: same program, per-core data)."""
    nc = bacc.Bacc()
    qb = min(512, lq)            # query block for matmul moving operand
    nq = lq // qb                # query blocks
    k16 = lk // 128              # 128-row key chunks

    # ---- DRAM I/O ----
    qT_d = nc.dram_tensor("qT", [E, lq], F32, kind="ExternalInput")
    kT_d = nc.dram_tensor("kT", [E, lk], F32, kind="ExternalInput")
    vT_d = nc.dram_tensor("vT", [E, lk], F32, kind="ExternalInput")
    maskT_d = nc.dram_tensor("maskT", [lk, lq], I32, kind="ExternalInput")
    geneT_d = nc.dram_tensor("geneT", [lk, lq], F32, kind="ExternalInput")
    Wq_d = nc.dram_tensor("Wq", [E, E], F32, kind="ExternalInput")
    Wk_d = nc.dram_tensor("Wk", [E, E], F32, kind="ExternalInput")
    Wv_d = nc.dram_tensor("Wv", [E, E], F32, kind="ExternalInput")
    Wo_d = nc.dram_tensor("Wo", [E, E], F32, kind="ExternalInput")
    bq_d = nc.dram_tensor("bq", [E], F32, kind="ExternalInput")
    bk_d = nc.dram_tensor("bk", [E], F32, kind="ExternalInput")
    bv_d = nc.dram_tensor("bv", [E], F32, kind="ExternalInput")
    bo_d = nc.dram_tensor("bo", [E], F32, kind="ExternalInput")
    attnwT_d = nc.dram_tensor("attnwT", [H, lk, lq], F32, kind="ExternalOutput")
    outT_d = nc.dram_tensor("outT", [E, lq], F32, kind="ExternalOutput")

    # DRAM view with 128-partition layout
    att_r = attnwT_d.rearrange("h (n p) q -> h p n q", p=128)  # [H,128,k16,lq]

    with tile.TileContext(nc) as tc, ExitStack() as ctx:
        const = ctx.enter_context(tc.tile_pool(name="const", bufs=1))
        persist = ctx.enter_context(tc.tile_pool(name="persist", bufs=1))

        ident16 = const.tile([128, 128], F16, tag="id16")
        make_identity(nc, ident16)
        ones128 = const.tile([1, 128], F32, tag="ones")
        nc.vector.memset(ones128, 1.0)
        bv_row = const.tile([1, E], F32, tag="bvrow")
        nc.sync.dma_start(out=bv_row, in_=bv_d.rearrange("(o e) -> o e", o=1))
        bq_s = const.tile([128, 4], F32, tag="bq")
        nc.sync.dma_start(out=bq_s, in_=bq_d.rearrange("(c p) -> p c", p=128))
        nc.scalar.mul(out=bq_s, in_=bq_s, mul=0.125)  # fold 1/sqrt(dh) into q
        bk_s = const.tile([128, 4], F32, tag="bk")
        nc.sync.dma_start(out=bk_s, in_=bk_d.rearrange("(c p) -> p c", p=128))
        bo_s = const.tile([128, 4], F32, tag="bo")
        nc.sync.dma_start(out=bo_s, in_=bo_d.rearrange("(c p) -> p c", p=128))
        eshift = const.tile([128, 1], F32, tag="eshift")
        nc.vector.memset(eshift, ESHIFT)

        # bf16 hi/lo splits of scaled-q^T and k^T, per-head V (+ones col),
        # fp16 masked gene bias, bf16 attention output
        Qh = persist.tile([128, 4, lq], BF16, tag="Qh")
        Ql = persist.tile([128, 4, lq], BF16, tag="Ql")
        Kh = persist.tile([128, 4, lk], BF16, tag="Kh")
        Kl = persist.tile([128, 4, lk], BF16, tag="Kl")
        V2 = persist.tile([128, k16, H, DH + 1], BF16, tag="V2")
        SBT = persist.tile([128, k16, lq], F16, tag="SBT")
        OTb = persist.tile([128, 4, lq], BF16, tag="OTb")
        Woh = persist.tile([128, 4, E], BF16, tag="Woh")

        nc.gpsimd.memset(V2[:, :, :, DH:DH + 1], 1.0)  # denominator ones-col

        # ---------- Phase 1: projections ----------
        with tc.tile_pool(name="ph1x", bufs=1) as xpool, \
             tc.tile_pool(name="ph1xs", bufs=1) as xspool, \
             tc.tile_pool(name="ph1w", bufs=1) as wpool, \
             tc.tile_pool(name="ph1t", bufs=4) as tpool, \
             tc.tile_pool(name="ph1ps", bufs=4, space="PSUM") as pp:

            Wof = wpool.tile([128, 4, E], F32, tag="wo_f")
            nc.sync.dma_start(out=Wof, in_=Wo_d.rearrange("(c p) e -> p c e", p=128))
            nc.vector.tensor_copy(out=Woh, in_=Wof)

            def load_split_x(x_dram, xlen, want_lo):
                """Load x^T [E, xlen] f32 and split to bf16 hi (+ lo)."""
                xT = xpool.tile([128, 4, xlen], F32, tag="x")
                nc.sync.dma_start(out=xT, in_=x_dram.rearrange("(c p) n -> p c n", p=128))
                xh = xspool.tile([128, 4, xlen], BF16, tag="xh")
                nc.vector.tensor_copy(out=xh, in_=xT)
                xl = None
                if want_lo:
                    xl = xspool.tile([128, 4, xlen], BF16, tag="xl")
                    nc.vector.tensor_sub(out=xl, in0=xT, in1=xh)
                return xh, xl

            def load_split_w(w_dram, want_lo):
                wf = wpool.tile([128, 4, E], F32, tag="w")
                nc.sync.dma_start(out=wf, in_=w_dram.rearrange("(c p) e -> p c e", p=128))
                wh = wpool.tile([128, 4, E], BF16, tag="wh")
                nc.vector.tensor_copy(out=wh, in_=wf)
                wl = None
                if want_lo:
                    wl = wpool.tile([128, 4, E], BF16, tag="wl")
                    nc.vector.tensor_sub(out=wl, in0=wf, in1=wh)
                return wh, wl

            def proj3(ps, wh, wl, xh, xl, m, sl):
                """ps += W^T x^T over 4 E-chunks, 3-term bf16 split."""
                for c in range(4):
                    first = c == 0
                    nc.tensor.matmul(ps, lhsT=wh[:, c, m * 128:(m + 1) * 128],
                                     rhs=xh[:, c, sl], start=first, stop=False)
                    nc.tensor.matmul(ps, lhsT=wh[:, c, m * 128:(m + 1) * 128],
                                     rhs=xl[:, c, sl], start=False, stop=False)
                    nc.tensor.matmul(ps, lhsT=wl[:, c, m * 128:(m + 1) * 128],
                                     rhs=xh[:, c, sl], start=False,
                                     stop=(c == 3))

            # Q^T (scaled 1/8, bias bq/8) -> split to Qh/Ql
            xh, xl = load_split_x(qT_d, lq, True)
            wh, wl = load_split_w(Wq_d, True)
            for m in range(4):
                for qc in range(nq):
                    sl = slice(qc * qb, (qc + 1) * qb)
                    ps = pp.tile([128, qb], F32, tag="ps")
                    proj3(ps, wh, wl, xh, xl, m, sl)
                    t = tpool.tile([128, qb], F32, tag="t")
                    nc.scalar.activation(out=t, in_=ps, func=AF.Identity,
                                         bias=bq_s[:, m:m + 1], scale=0.125)
                    nc.vector.tensor_copy(out=Qh[:, m, sl], in_=t)
                    nc.vector.tensor_sub(out=Ql[:, m, sl], in0=t, in1=Qh[:, m, sl])
            # K^T -> split to Kh/Kl
            xh, xl = load_split_x(kT_d, lk, True)
            wh, wl = load_split_w(Wk_d, True)
            for m in range(4):
                for qc in range(lk // 512):
                    sl = slice(qc * 512, (qc + 1) * 512)
                    ps = pp.tile([128, 512], F32, tag="ps")
                    proj3(ps, wh, wl, xh, xl, m, sl)
                    t = tpool.tile([128, 512], F32, tag="t")
                    nc.scalar.activation(out=t, in_=ps, func=AF.Identity,
                                         bias=bk_s[:, m:m + 1], scale=1.0)
                    nc.vector.tensor_copy(out=Kh[:, m, sl], in_=t)
                    nc.vector.tensor_sub(out=Kl[:, m, sl], in0=t, in1=Kh[:, m, sl])
            # V (plain bf16; precision non-critical): natural [k, dh] + bias
            xh, _ = load_split_x(vT_d, lk, False)
            wh, _ = load_split_w(Wv_d, False)
            for kc in range(k16):
                ps = pp.tile([128, E], F32, tag="psv")
                for c in range(4):
                    nc.tensor.matmul(ps, lhsT=xh[:, c, kc * 128:(kc + 1) * 128],
                                     rhs=wh[:, c, :], start=(c == 0), stop=False)
                nc.tensor.matmul(ps, lhsT=ones128, rhs=bv_row,
                                 start=False, stop=True)  # += bv broadcast
                nc.scalar.activation(
                    out=V2[:, kc, :, 0:DH],
                    in_=ps.rearrange("p (h d) -> p h d", h=H),
                    func=AF.Copy)

        # ---------- Phase 2a: masked gene bias (transposed), fp16 ----------
        with tc.tile_pool(name="ph2a", bufs=2) as mg:
            for kc in range(k16):
                mt = mg.tile([128, lq], I32, tag="m")
                nc.sync.dma_start(out=mt, in_=maskT_d[kc * 128:(kc + 1) * 128, :])
                gt = mg.tile([128, lq], F32, tag="g")
                nc.sync.dma_start(out=gt, in_=geneT_d[kc * 128:(kc + 1) * 128, :])
                gf = mg.tile([128, lq], F16, tag="gf")
                nc.vector.tensor_copy(out=gf, in_=gt)     # f32 -> f16
                nc.gpsimd.memset(SBT[:, kc, :], NEG)
                nc.vector.copy_predicated(out=SBT[:, kc, :], mask=mt, data=gf)

        # ---------- Phase 2b: attention ----------
        with tc.tile_pool(name="spsum", bufs=5, space="PSUM") as spsum, \
             tc.tile_pool(name="opsum", bufs=2, space="PSUM") as opsum, \
             tc.tile_pool(name="strip", bufs=2) as strip, \
             tc.tile_pool(name="pout", bufs=3) as pout, \
             tc.tile_pool(name="small", bufs=4) as small:
            for h in range(H):
                hp, hc = (h % 2) * 64, h // 2
                for qg in range(nq):
                    qs = slice(qg * qb, (qg + 1) * qb)
                    Pt = strip.tile([128, k16, qb], BF16, tag="pt")
                    for kc in range(k16):
                        ks = slice(kc * 128, (kc + 1) * 128)
                        ps = spsum.tile([128, qb], F32, tag="s")
                        # S^T[k,q] = K_h Q_h^T, 3-term bf16 split
                        nc.tensor.matmul(ps, lhsT=Kh[hp:hp + 64, hc, ks],
                                         rhs=Qh[hp:hp + 64, hc, qs],
                                         start=True, stop=False)
                        nc.tensor.matmul(ps, lhsT=Kh[hp:hp + 64, hc, ks],
                                         rhs=Ql[hp:hp + 64, hc, qs],
                                         start=False, stop=False)
                        nc.tensor.matmul(ps, lhsT=Kl[hp:hp + 64, hc, ks],
                                         rhs=Qh[hp:hp + 64, hc, qs],
                                         start=False, stop=False)
                        # += gene/mask bias via identity weights (fp16)
                        nc.tensor.matmul(ps, lhsT=ident16, rhs=SBT[:, kc, qs],
                                         start=False, stop=True)
                        nc.scalar.activation(out=Pt[:, kc, :], in_=ps,
                                             func=AF.Exp, bias=eshift, scale=1.0)
                    # O'^T[dh,q] (+ denominator row) = [V_h|1]^T P~^T
                    ops = opsum.tile([DH + 1, qb], F32, tag="o")
                    for kc in range(k16):
                        nc.tensor.matmul(ops, lhsT=V2[:, kc, h, :], rhs=Pt[:, kc, :],
                                         start=(kc == 0), stop=(kc == k16 - 1))
                    rd = small.tile([1, qb], F32, tag="rd")
                    nc.vector.reciprocal(out=rd, in_=ops[DH:DH + 1, :])
                    # broadcast 1/denom across partitions on GpSimd
                    # (keeps the PE stream free of the reciprocal dependency)
                    rb = small.tile([128, qb], F32, tag="rbs")
                    nc.gpsimd.partition_broadcast(rb, rd)
                    # normalized attention out^T rows for this head (bf16)
                    nc.vector.tensor_mul(out=OTb[hp:hp + 64, hc, qs],
                                         in0=ops[0:DH, :], in1=rb[0:DH, :])
                    # normalized weights -> DRAM f32, 4 key-chunks per DMA
                    for c4 in range(k16 // 4):
                        Pn = pout.tile([128, 4, qb], F32, tag="pn")
                        for j in range(4):
                            nc.vector.tensor_mul(out=Pn[:, j, :],
                                                 in0=Pt[:, c4 * 4 + j, :], in1=rb)
                        nc.sync.dma_start(
                            out=att_r[h, :, c4 * 4:(c4 + 1) * 4, qs], in_=Pn)

        # ---------- Phase 3: output projection (bf16) ----------
        with tc.tile_pool(name="ph3ps", bufs=2, space="PSUM") as wp, \
             tc.tile_pool(name="ph3y", bufs=2) as yp:
            for qc in range(nq):
                for m in range(4):
                    ps = wp.tile([128, qb], F32, tag="y")
                    for c in range(4):
                        nc.tensor.matmul(ps, lhsT=Woh[:, c, m * 128:(m + 1) * 128],
                                         rhs=OTb[:, c, qc * qb:(qc + 1) * qb],
                                         start=(c == 0), stop=(c == 3))
                    y = yp.tile([128, qb], F32, tag="ysb")
                    nc.scalar.activation(out=y, in_=ps, func=AF.Identity,
                                         bias=bo_s[:, m:m + 1], scale=1.0)
                    nc.sync.dma_start(out=outT_d[m * 128:(m + 1) * 128,
                                                 qc * qb:(qc + 1) * qb], in_=y)

    nc.finalize()
    return nc


_NC_CACHE = {}


def _get_nc():
    if "nc" not in _NC_CACHE:
        _NC_CACHE["nc"] = build_nc()
    return _NC_CACHE["nc"]


def make_in_maps(query, key, value, attn_mask, gene_regulatory_matrix,
                 Wq, bq, Wk, bk, Wv, bv, Wo, bo):
    f = lambda a: np.asarray(a, np.float32)
    query, key, value = f(query), f(key), f(value)
    gene = f(gene_regulatory_matrix)
    mask = np.asarray(attn_mask, np.int32)
    shared = {"Wq": f(Wq), "Wk": f(Wk), "Wv": f(Wv), "Wo": f(Wo),
              "bq": f(bq), "bk": f(bk), "bv": f(bv), "bo": f(bo)}
    in_maps = []
    for c in range(NCORES):
        b, half = c // 2, c % 2
        qs = slice(half * LQ, (half + 1) * LQ)
        in_maps.append({
            "qT": np.ascontiguousarray(query[b, qs].T),
            "kT": np.ascontiguousarray(key[b].T),
            "vT": np.ascontiguousarray(value[b].T),
            "maskT": np.ascontiguousarray(mask[b, qs].T),
            "geneT": np.ascontiguousarray(gene[qs].T),
            **shared,
        })
    return in_maps


def assemble(results):
    out = np.empty((B, L, E), np.float32)
    aw = np.empty((B, H, L, L), np.float32)
    for c in range(NCORES):
        b, half = c // 2, c % 2
        qs = slice(half * LQ, (half + 1) * LQ)
        aw[b, :, qs, :] = results[c]["attnwT"].transpose(0, 2, 1)
        out[b, qs, :] = results[c]["outT"].T
    return out, aw


def kernel(query, key, value, attn_mask, gene_regulatory_matrix,
           Wq, bq, Wk, bk, Wv, bv, Wo, bo):
    nc = _get_nc()
    in_maps = make_in_maps(query, key, value, attn_mask,
                           gene_regulatory_matrix,
                           Wq, bq, Wk, bk, Wv, bv, Wo, bo)
    res = run_bass_kernel_spmd(nc, in_maps, list(range(NCORES))).results
    return assemble(res)


# revision 15
# speedup vs baseline: 2.1236x; 1.1092x over previous
"""Trainium2 Bass kernel for MultiHeadAttention with gene-regulatory bias.

Reference computation (per batch b):
    q,k,v = (x @ W + b) split into 8 heads of 64
    scores = q k^T / 8 + gene[None]        (gene shared across batch/heads)
    scores = where(mask==0, -inf, scores)
    P = softmax(scores, axis=-1)           -> returned as attention_weights
    out = (P v) concat-heads @ Wo + bo

Sharding: 8 cores = (batch b, query-half). Each core handles 1024 query rows
x full 2048 keys x all 8 heads. No collectives; host reassembles.

Device dataflow (per core) is "transposed-score" oriented: S^T[k,q] tiles are
computed natively on the PE (so the P@V contraction needs no on-chip transpose
of the big score tensor), the softmax denominator is obtained by appending a
ones-column to V (one extra PSUM row in the P@V matmul), and normalization
happens along the free axis on the vector engine. The [k,q] -> [q,k] layout
fix of the returned weights is done on the host (pure layout/unshard step).

Numerics: fp32 matmuls run in the PE's slow LOW_HIGH mode, so the whole
pipeline is fp16 (x/W shards are cast to fp16 on the host; the 0/1 mask is
int8). fp16 keeps ~5e-4 relative accuracy per tensor and ~4e-3 end to end,
comfortably inside the absmax gate. exp is shifted by -4 (softmax-invariant
range guard); all matmul accumulation is fp32 PSUM; softmax normalization and
both outputs stay fp32.
"""

import numpy as np
from contextlib import ExitStack

import concourse.bass as bass
import concourse.bacc as bacc
import concourse.tile as tile
from concourse import mybir
from concourse.masks import make_identity
from concourse.bass_utils import run_bass_kernel_spmd

F32 = mybir.dt.float32
F16 = mybir.dt.float16
I8 = mybir.dt.int8
AF = mybir.ActivationFunctionType

B, L, E = 4, 2048, 512
H, DH = 8, 64
NCORES = 8
LQ = L // 2          # query rows per core
LK = L
NEG = -10000.0       # "minus infinity" for masked scores
ESHIFT = -4.0        # exp(s + ESHIFT): softmax-invariant fp16 range guard


def build_nc(lq=LQ, lk=LK):
    """Build the per-core Bass program (SPMD: same program, per-core data)."""
    nc = bacc.Bacc()
    qb = min(512, lq)            # query block for matmul moving operand
    nq = lq // qb                # query blocks
    k16 = lk // 128              # 128-row key chunks

    # ---- DRAM I/O (x/W pre-cast to fp16 on host; mask int8) ----
    qT_d = nc.dram_tensor("qT", [E, lq], F16, kind="ExternalInput")
    kT_d = nc.dram_tensor("kT", [E, lk], F16, kind="ExternalInput")
    vT_d = nc.dram_tensor("vT", [E, lk], F16, kind="ExternalInput")
    maskT_d = nc.dram_tensor("maskT", [lk, lq], I8, kind="ExternalInput")
    geneT_d = nc.dram_tensor("geneT", [lk, lq], F16, kind="ExternalInput")
    Wq_d = nc.dram_tensor("Wq", [E, E], F16, kind="ExternalInput")
    Wk_d = nc.dram_tensor("Wk", [E, E], F16, kind="ExternalInput")
    Wv_d = nc.dram_tensor("Wv", [E, E], F16, kind="ExternalInput")
    Wo_d = nc.dram_tensor("Wo", [E, E], F16, kind="ExternalInput")
    bq_d = nc.dram_tensor("bq", [E], F32, kind="ExternalInput")
    bk_d = nc.dram_tensor("bk", [E], F32, kind="ExternalInput")
    bv_d = nc.dram_tensor("bv", [E], F32, kind="ExternalInput")
    bo_d = nc.dram_tensor("bo", [E], F32, kind="ExternalInput")
    attnwT_d = nc.dram_tensor("attnwT", [H, lk, lq], F32, kind="ExternalOutput")
    outT_d = nc.dram_tensor("outT", [E, lq], F32, kind="ExternalOutput")

    # DRAM view with 128-partition layout
    att_r = attnwT_d.rearrange("h (n p) q -> h p n q", p=128)  # [H,128,k16,lq]

    with tile.TileContext(nc) as tc, ExitStack() as ctx:
        const = ctx.enter_context(tc.tile_pool(name="const", bufs=1))
        persist = ctx.enter_context(tc.tile_pool(name="persist", bufs=1))

        ident16 = const.tile([128, 128], F16, tag="id16")
        make_identity(nc, ident16)
        ones16 = const.tile([1, 128], F16, tag="ones")
        nc.vector.memset(ones16, 1.0)
        bv_row = const.tile([1, E], F16, tag="bvrow")
        nc.gpsimd.dma_start(out=bv_row, in_=bv_d.rearrange("(o e) -> o e", o=1))
        bq_s = const.tile([128, 4], F32, tag="bq")
        nc.sync.dma_start(out=bq_s, in_=bq_d.rearrange("(c p) -> p c", p=128))
        nc.scalar.mul(out=bq_s, in_=bq_s, mul=0.125)  # fold 1/sqrt(dh) into q
        bk_s = const.tile([128, 4], F32, tag="bk")
        nc.sync.dma_start(out=bk_s, in_=bk_d.rearrange("(c p) -> p c", p=128))
        bo_s = const.tile([128, 4], F32, tag="bo")
        nc.sync.dma_start(out=bo_s, in_=bo_d.rearrange("(c p) -> p c", p=128))
        eshift = const.tile([128, 1], F32, tag="eshift")
        nc.vector.memset(eshift, ESHIFT)

        Qf = persist.tile([128, 4, lq], F16, tag="Qf")     # q^T/8+bq/8 [E,lq]
        Kf = persist.tile([128, 4, lk], F16, tag="Kf")     # k^T        [E,lk]
        V2 = persist.tile([128, k16, H, DH + 1], F16, tag="V2")  # [V_h | 1]
        SBT = persist.tile([128, k16, lq], F16, tag="SBT")  # masked gene^T
        OTb = persist.tile([128, 4, lq], F16, tag="OTb")   # attn out^T [E,lq]
        Wo16 = persist.tile([128, 4, E], F16, tag="Wo16")
        nc.sync.dma_start(out=Wo16, in_=Wo_d.rearrange("(c p) e -> p c e", p=128))

        nc.gpsimd.memset(V2[:, :, :, DH:DH + 1], 1.0)  # denominator ones-col

        # ---------- Phase 1: projections (all fp16 operands) ----------
        with tc.tile_pool(name="ph1x", bufs=2) as xpool, \
             tc.tile_pool(name="ph1w", bufs=2) as wpool, \
             tc.tile_pool(name="ph1ps", bufs=4, space="PSUM") as pp:

            def load_xw(x_dram, w_dram, xlen):
                xT = xpool.tile([128, 4, xlen], F16, tag="x")
                nc.sync.dma_start(out=xT, in_=x_dram.rearrange("(c p) n -> p c n", p=128))
                W = wpool.tile([128, 4, E], F16, tag="w")
                nc.sync.dma_start(out=W, in_=w_dram.rearrange("(c p) e -> p c e", p=128))
                return xT, W

            # Q^T = Wq^T x^T (scaled 1/8, bias bq/8)
            xT, W = load_xw(qT_d, Wq_d, lq)
            for m in range(4):
                for qc in range(nq):
                    sl = slice(qc * qb, (qc + 1) * qb)
                    ps = pp.tile([128, qb], F32, tag="ps")
                    for c in range(4):
                        nc.tensor.matmul(ps, lhsT=W[:, c, m * 128:(m + 1) * 128],
                                         rhs=xT[:, c, sl],
                                         start=(c == 0), stop=(c == 3))
                    nc.scalar.activation(out=Qf[:, m, sl], in_=ps, func=AF.Identity,
                                         bias=bq_s[:, m:m + 1], scale=0.125)
            # K^T = Wk^T x^T
            xT, W = load_xw(kT_d, Wk_d, lk)
            for m in range(4):
                for qc in range(lk // 512):
                    sl = slice(qc * 512, (qc + 1) * 512)
                    ps = pp.tile([128, 512], F32, tag="ps")
                    for c in range(4):
                        nc.tensor.matmul(ps, lhsT=W[:, c, m * 128:(m + 1) * 128],
                                         rhs=xT[:, c, sl],
                                         start=(c == 0), stop=(c == 3))
                    nc.scalar.activation(out=Kf[:, m, sl], in_=ps, func=AF.Identity,
                                         bias=bk_s[:, m:m + 1], scale=1.0)
            # V = value Wv + bv (natural [k, dh] layout, per-head + ones col)
            xT, W = load_xw(vT_d, Wv_d, lk)
            for kc in range(k16):
                ps = pp.tile([128, E], F32, tag="psv")
                for c in range(4):
                    nc.tensor.matmul(ps, lhsT=xT[:, c, kc * 128:(kc + 1) * 128],
                                     rhs=W[:, c, :], start=(c == 0), stop=False)
                nc.tensor.matmul(ps, lhsT=ones16, rhs=bv_row,
                                 start=False, stop=True)  # += bv broadcast
                nc.scalar.activation(
                    out=V2[:, kc, :, 0:DH],
                    in_=ps.rearrange("p (h d) -> p h d", h=H),
                    func=AF.Copy)

        # ---------- Phase 2a: masked gene bias (transposed), fp16 ----------
        with tc.tile_pool(name="ph2a", bufs=2) as mg:
            for kc in range(k16):
                mt = mg.tile([128, lq], I8, tag="m")
                nc.sync.dma_start(out=mt, in_=maskT_d[kc * 128:(kc + 1) * 128, :])
                gt = mg.tile([128, lq], F16, tag="g")
                nc.sync.dma_start(out=gt, in_=geneT_d[kc * 128:(kc + 1) * 128, :])
                nc.gpsimd.memset(SBT[:, kc, :], NEG)
                nc.vector.copy_predicated(out=SBT[:, kc, :], mask=mt, data=gt)

        # ---------- Phase 2b: attention ----------
        with tc.tile_pool(name="spsum", bufs=3, space="PSUM") as spsum, \
             tc.tile_pool(name="opsum", bufs=2, space="PSUM") as opsum, \
             tc.tile_pool(name="strip", bufs=2) as strip, \
             tc.tile_pool(name="pout", bufs=3) as pout, \
             tc.tile_pool(name="small", bufs=4) as small:
            for h in range(H):
                hp, hc = (h % 2) * 64, h // 2
                for qg in range(nq):
                    qs = slice(qg * qb, (qg + 1) * qb)
                    Pt = strip.tile([128, k16, qb], F16, tag="pt")
                    for kt in range(k16 // 2):
                        ps = spsum.tile([128, 2, qb], F32, tag="s")
                        for j in range(2):
                            kc = 2 * kt + j
                            ks = slice(kc * 128, (kc + 1) * 128)
                            # S^T[k,q] = K_h Q_h^T  (q pre-scaled by 1/8)
                            nc.tensor.matmul(ps[:, j, :],
                                             lhsT=Kf[hp:hp + 64, hc, ks],
                                             rhs=Qf[hp:hp + 64, hc, qs],
                                             start=True, stop=False)
                            # += gene/mask bias via identity weights
                            nc.tensor.matmul(ps[:, j, :], lhsT=ident16,
                                             rhs=SBT[:, kc, qs],
                                             start=False, stop=True)
                        nc.scalar.activation(out=Pt[:, 2 * kt:2 * kt + 2, :],
                                             in_=ps, func=AF.Exp,
                                             bias=eshift, scale=1.0)
                    # O'^T[dh,q] (+ denominator row) = [V_h|1]^T P~^T
                    ops = opsum.tile([DH + 1, qb], F32, tag="o")
                    for kc in range(k16):
                        nc.tensor.matmul(ops, lhsT=V2[:, kc, h, :], rhs=Pt[:, kc, :],
                                         start=(kc == 0), stop=(kc == k16 - 1))
                    rd = small.tile([1, qb], F32, tag="rd")
                    nc.vector.reciprocal(out=rd, in_=ops[DH:DH + 1, :])
                    # broadcast 1/denom across partitions on GpSimd
                    # (keeps the PE stream free of the reciprocal dependency)
                    rb = small.tile([128, qb], F32, tag="rbs")
                    nc.gpsimd.partition_broadcast(rb, rd)
                    # normalized attention out^T rows for this head (fp16)
                    nc.vector.tensor_mul(out=OTb[hp:hp + 64, hc, qs],
                                         in0=ops[0:DH, :], in1=rb[0:DH, :])
                    # normalized weights -> DRAM f32, 4 key-chunks per DMA
                    for c4 in range(k16 // 4):
                        Pn = pout.tile([128, 4, qb], F32, tag="pn")
                        for j in range(4):
                            nc.vector.tensor_mul(out=Pn[:, j, :],
                                                 in0=Pt[:, c4 * 4 + j, :], in1=rb)
                        nc.sync.dma_start(
                            out=att_r[h, :, c4 * 4:(c4 + 1) * 4, qs], in_=Pn)

        # ---------- Phase 3: output projection (fp16) ----------
        with tc.tile_pool(name="ph3ps", bufs=2, space="PSUM") as wp, \
             tc.tile_pool(name="ph3y", bufs=2) as yp:
            for qc in range(nq):
                for m in range(4):
                    ps = wp.tile([128, qb], F32, tag="y")
                    for c in range(4):
                        nc.tensor.matmul(ps, lhsT=Wo16[:, c, m * 128:(m + 1) * 128],
                                         rhs=OTb[:, c, qc * qb:(qc + 1) * qb],
                                         start=(c == 0), stop=(c == 3))
                    y = yp.tile([128, qb], F32, tag="ysb")
                    nc.scalar.activation(out=y, in_=ps, func=AF.Identity,
                                         bias=bo_s[:, m:m + 1], scale=1.0)
                    nc.sync.dma_start(out=outT_d[m * 128:(m + 1) * 128,
                                                 qc * qb:(qc + 1) * qb], in_=y)

    nc.finalize()
    return nc


_NC_CACHE = {}


def _get_nc():
    if "nc" not in _NC_CACHE:
        _NC_CACHE["nc"] = build_nc()
    return _NC_CACHE["nc"]


def make_in_maps(query, key, value, attn_mask, gene_regulatory_matrix,
                 Wq, bq, Wk, bk, Wv, bv, Wo, bo):
    h = lambda a: np.asarray(a, np.float16)
    f = lambda a: np.asarray(a, np.float32)
    query, key, value = h(query), h(key), h(value)
    gene = h(gene_regulatory_matrix)
    mask = np.asarray(attn_mask).astype(np.int8)
    shared = {"Wq": h(Wq), "Wk": h(Wk), "Wv": h(Wv), "Wo": h(Wo),
              "bq": f(bq), "bk": f(bk), "bv": f(bv), "bo": f(bo)}
    in_maps = []
    for c in range(NCORES):
        b, half = c // 2, c % 2
        qs = slice(half * LQ, (half + 1) * LQ)
        in_maps.append({
            "qT": np.ascontiguousarray(query[b, qs].T),
            "kT": np.ascontiguousarray(key[b].T),
            "vT": np.ascontiguousarray(value[b].T),
            "maskT": np.ascontiguousarray(mask[b, qs].T),
            "geneT": np.ascontiguousarray(gene[qs].T),
            **shared,
        })
    return in_maps


def assemble(results):
    out = np.empty((B, L, E), np.float32)
    aw = np.empty((B, H, L, L), np.float32)
    for c in range(NCORES):
        b, half = c // 2, c % 2
        qs = slice(half * LQ, (half + 1) * LQ)
        aw[b, :, qs, :] = results[c]["attnwT"].transpose(0, 2, 1)
        out[b, qs, :] = results[c]["outT"].T
    return out, aw


def kernel(query, key, value, attn_mask, gene_regulatory_matrix,
           Wq, bq, Wk, bk, Wv, bv, Wo, bo):
    nc = _get_nc()
    in_maps = make_in_maps(query, key, value, attn_mask,
                           gene_regulatory_matrix,
                           Wq, bq, Wk, bk, Wv, bv, Wo, bo)
    res = run_bass_kernel_spmd(nc, in_maps, list(range(NCORES))).results
    return assemble(res)


# revision 17
# speedup vs baseline: 2.3932x; 1.1270x over previous
"""Trainium2 Bass kernel for MultiHeadAttention with gene-regulatory bias.

Reference computation (per batch b):
    q,k,v = (x @ W + b) split into 8 heads of 64
    scores = q k^T / 8 + gene[None]        (gene shared across batch/heads)
    scores = where(mask==0, -inf, scores)
    P = softmax(scores, axis=-1)           -> returned as attention_weights
    out = (P v) concat-heads @ Wo + bo

Sharding: 8 cores = (batch b, query-half). Each core handles 1024 query rows
x full 2048 keys x all 8 heads. No collectives; host reassembles.

Device dataflow (per core) is "transposed-score" oriented: S^T[k,q] tiles are
computed natively on the PE (so the P@V contraction needs no on-chip transpose
of the big score tensor), the softmax denominator is obtained by appending a
ones-column to V (one extra PSUM row in the P@V matmul), and normalization
happens along the free axis on the vector engine. The [k,q] -> [q,k] layout
fix of the returned weights is done on the host (pure layout/unshard step).

Numerics: fp32 matmuls run in the PE's slow LOW_HIGH mode, so the whole
pipeline is fp16 (x/W shards are cast to fp16 on the host; the 0/1 mask is
int8). fp16 keeps ~5e-4 relative accuracy per tensor and ~4e-3 end to end,
comfortably inside the absmax gate. exp is shifted by -4 (softmax-invariant
range guard); all matmul accumulation is fp32 PSUM; softmax normalization and
both outputs stay fp32.
"""

import numpy as np
from contextlib import ExitStack

import concourse.bass as bass
import concourse.bacc as bacc
import concourse.tile as tile
from concourse import mybir
from concourse.masks import make_identity
from concourse.bass_utils import run_bass_kernel_spmd

F32 = mybir.dt.float32
F16 = mybir.dt.float16
I8 = mybir.dt.int8
AF = mybir.ActivationFunctionType

B, L, E = 4, 2048, 512
H, DH = 8, 64
NCORES = 8
LQ = L // 2          # query rows per core
LK = L
NEG = -10000.0       # "minus infinity" for masked scores
ESHIFT = -4.0        # exp(s + ESHIFT): softmax-invariant fp16 range guard


def build_nc(lq=LQ, lk=LK):
    """Build the per-core Bass program (SPMD: same program, per-core data)."""
    nc = bacc.Bacc()
    qb = min(512, lq)            # query block for matmul moving operand
    nq = lq // qb                # query blocks
    k16 = lk // 128              # 128-row key chunks

    # ---- DRAM I/O (x/W pre-cast to fp16 on host; mask int8) ----
    qT_d = nc.dram_tensor("qT", [E, lq], F16, kind="ExternalInput")
    kT_d = nc.dram_tensor("kT", [E, lk], F16, kind="ExternalInput")
    vT_d = nc.dram_tensor("vT", [E, lk], F16, kind="ExternalInput")
    maskT_d = nc.dram_tensor("maskT", [lk, lq], I8, kind="ExternalInput")
    geneT_d = nc.dram_tensor("geneT", [lk, lq], F16, kind="ExternalInput")
    Wq_d = nc.dram_tensor("Wq", [E, E], F16, kind="ExternalInput")
    Wk_d = nc.dram_tensor("Wk", [E, E], F16, kind="ExternalInput")
    Wv_d = nc.dram_tensor("Wv", [E, E], F16, kind="ExternalInput")
    Wo_d = nc.dram_tensor("Wo", [E, E], F16, kind="ExternalInput")
    bq_d = nc.dram_tensor("bq", [E], F32, kind="ExternalInput")
    bk_d = nc.dram_tensor("bk", [E], F32, kind="ExternalInput")
    bv_d = nc.dram_tensor("bv", [E], F32, kind="ExternalInput")
    bo_d = nc.dram_tensor("bo", [E], F32, kind="ExternalInput")
    attnwT_d = nc.dram_tensor("attnwT", [H, lk, lq], F32, kind="ExternalOutput")
    outT_d = nc.dram_tensor("outT", [E, lq], F32, kind="ExternalOutput")

    # DRAM view with 128-partition layout
    att_r = attnwT_d.rearrange("h (n p) q -> h p n q", p=128)  # [H,128,k16,lq]

    with tile.TileContext(nc) as tc, ExitStack() as ctx:
        const = ctx.enter_context(tc.tile_pool(name="const", bufs=1))
        persist = ctx.enter_context(tc.tile_pool(name="persist", bufs=1))

        ident16 = const.tile([128, 128], F16, tag="id16")
        make_identity(nc, ident16)
        ones16 = const.tile([1, 128], F16, tag="ones")
        nc.vector.memset(ones16, 1.0)
        bv_row = const.tile([1, E], F16, tag="bvrow")
        nc.gpsimd.dma_start(out=bv_row, in_=bv_d.rearrange("(o e) -> o e", o=1))
        bq_s = const.tile([128, 4], F32, tag="bq")
        nc.sync.dma_start(out=bq_s, in_=bq_d.rearrange("(c p) -> p c", p=128))
        nc.scalar.mul(out=bq_s, in_=bq_s, mul=0.125)  # fold 1/sqrt(dh) into q
        bk_s = const.tile([128, 4], F32, tag="bk")
        nc.sync.dma_start(out=bk_s, in_=bk_d.rearrange("(c p) -> p c", p=128))
        bo_s = const.tile([128, 4], F32, tag="bo")
        nc.sync.dma_start(out=bo_s, in_=bo_d.rearrange("(c p) -> p c", p=128))
        eshift = const.tile([128, 1], F32, tag="eshift")
        nc.vector.memset(eshift, ESHIFT)

        Qf = persist.tile([128, 4, lq], F16, tag="Qf")     # q^T/8+bq/8 [E,lq]
        Kf = persist.tile([128, 4, lk], F16, tag="Kf")     # k^T        [E,lk]
        V2 = persist.tile([128, k16, H, DH + 1], F16, tag="V2")  # [V_h | 1]
        SBT = persist.tile([128, k16, lq], F16, tag="SBT")  # masked gene^T
        OTb = persist.tile([128, 4, lq], F16, tag="OTb")   # attn out^T [E,lq]
        Wo16 = persist.tile([128, 4, E], F16, tag="Wo16")
        nc.sync.dma_start(out=Wo16, in_=Wo_d.rearrange("(c p) e -> p c e", p=128))

        nc.gpsimd.memset(V2[:, :, :, DH:DH + 1], 1.0)  # denominator ones-col

        # ---------- Phase 1: projections (all fp16 operands) ----------
        with tc.tile_pool(name="ph1x", bufs=2) as xpool, \
             tc.tile_pool(name="ph1w", bufs=2) as wpool, \
             tc.tile_pool(name="ph1ps", bufs=4, space="PSUM") as pp:

            def load_xw(x_dram, w_dram, xlen):
                xT = xpool.tile([128, 4, xlen], F16, tag="x")
                nc.sync.dma_start(out=xT, in_=x_dram.rearrange("(c p) n -> p c n", p=128))
                W = wpool.tile([128, 4, E], F16, tag="w")
                nc.sync.dma_start(out=W, in_=w_dram.rearrange("(c p) e -> p c e", p=128))
                return xT, W

            # Q^T = Wq^T x^T (scaled 1/8, bias bq/8)
            xT, W = load_xw(qT_d, Wq_d, lq)
            for m in range(4):
                for qc in range(nq):
                    sl = slice(qc * qb, (qc + 1) * qb)
                    ps = pp.tile([128, qb], F32, tag="ps")
                    for c in range(4):
                        nc.tensor.matmul(ps, lhsT=W[:, c, m * 128:(m + 1) * 128],
                                         rhs=xT[:, c, sl],
                                         start=(c == 0), stop=(c == 3))
                    nc.scalar.activation(out=Qf[:, m, sl], in_=ps, func=AF.Identity,
                                         bias=bq_s[:, m:m + 1], scale=0.125)
            # K^T = Wk^T x^T
            xT, W = load_xw(kT_d, Wk_d, lk)
            for m in range(4):
                for qc in range(lk // 512):
                    sl = slice(qc * 512, (qc + 1) * 512)
                    ps = pp.tile([128, 512], F32, tag="ps")
                    for c in range(4):
                        nc.tensor.matmul(ps, lhsT=W[:, c, m * 128:(m + 1) * 128],
                                         rhs=xT[:, c, sl],
                                         start=(c == 0), stop=(c == 3))
                    nc.scalar.activation(out=Kf[:, m, sl], in_=ps, func=AF.Identity,
                                         bias=bk_s[:, m:m + 1], scale=1.0)
            # V = value Wv + bv (natural [k, dh] layout, per-head + ones col)
            xT, W = load_xw(vT_d, Wv_d, lk)
            for kc in range(k16):
                ps = pp.tile([128, E], F32, tag="psv")
                for c in range(4):
                    nc.tensor.matmul(ps, lhsT=xT[:, c, kc * 128:(kc + 1) * 128],
                                     rhs=W[:, c, :], start=(c == 0), stop=False)
                nc.tensor.matmul(ps, lhsT=ones16, rhs=bv_row,
                                 start=False, stop=True)  # += bv broadcast
                nc.scalar.activation(
                    out=V2[:, kc, :, 0:DH],
                    in_=ps.rearrange("p (h d) -> p h d", h=H),
                    func=AF.Copy)

        # ---------- Phase 2a: masked gene bias (transposed), fp16 ----------
        with tc.tile_pool(name="ph2a", bufs=2) as mg:
            for kc in range(k16):
                mt = mg.tile([128, lq], I8, tag="m")
                nc.sync.dma_start(out=mt, in_=maskT_d[kc * 128:(kc + 1) * 128, :])
                gt = mg.tile([128, lq], F16, tag="g")
                nc.sync.dma_start(out=gt, in_=geneT_d[kc * 128:(kc + 1) * 128, :])
                nc.gpsimd.memset(SBT[:, kc, :], NEG)
                nc.vector.copy_predicated(out=SBT[:, kc, :], mask=mt, data=gt)

        # ---------- Phase 2b: attention ----------
        with tc.tile_pool(name="spsum", bufs=3, space="PSUM") as spsum, \
             tc.tile_pool(name="opsum", bufs=2, space="PSUM") as opsum, \
             tc.tile_pool(name="strip", bufs=3) as strip, \
             tc.tile_pool(name="pout", bufs=4) as pout, \
             tc.tile_pool(name="small", bufs=4) as small:
            for h in range(H):
                hp, hc = (h % 2) * 64, h // 2
                for qg in range(nq):
                    qs = slice(qg * qb, (qg + 1) * qb)
                    Pt = strip.tile([128, k16, qb], F16, tag="pt")
                    for kt in range(k16 // 2):
                        ps = spsum.tile([128, 2, qb], F32, tag="s")
                        for j in range(2):
                            kc = 2 * kt + j
                            ks = slice(kc * 128, (kc + 1) * 128)
                            # S^T[k,q] = K_h Q_h^T  (q pre-scaled by 1/8)
                            nc.tensor.matmul(ps[:, j, :],
                                             lhsT=Kf[hp:hp + 64, hc, ks],
                                             rhs=Qf[hp:hp + 64, hc, qs],
                                             start=True, stop=False)
                            # += gene/mask bias via identity weights
                            nc.tensor.matmul(ps[:, j, :], lhsT=ident16,
                                             rhs=SBT[:, kc, qs],
                                             start=False, stop=True)
                        nc.scalar.activation(out=Pt[:, 2 * kt:2 * kt + 2, :],
                                             in_=ps, func=AF.Exp,
                                             bias=eshift, scale=1.0)
                    # O'^T[dh,q] (+ denominator row) = [V_h|1]^T P~^T
                    ops = opsum.tile([DH + 1, qb], F32, tag="o")
                    for kc in range(k16):
                        nc.tensor.matmul(ops, lhsT=V2[:, kc, h, :], rhs=Pt[:, kc, :],
                                         start=(kc == 0), stop=(kc == k16 - 1))
                    rd = small.tile([1, qb], F32, tag="rd")
                    nc.vector.reciprocal(out=rd, in_=ops[DH:DH + 1, :])
                    # broadcast 1/denom across partitions on GpSimd
                    # (keeps the PE stream free of the reciprocal dependency)
                    rb = small.tile([128, qb], F32, tag="rbs")
                    nc.gpsimd.partition_broadcast(rb, rd)
                    # normalized attention out^T rows for this head (fp16)
                    nc.vector.tensor_mul(out=OTb[hp:hp + 64, hc, qs],
                                         in0=ops[0:DH, :], in1=rb[0:DH, :])
                    # normalized weights -> DRAM f32, 4 key-chunks per DMA
                    rbap = rb[:, :]
                    rb4 = bass.AP(tensor=rbap.tensor, offset=rbap.offset,
                                  ap=[rbap.ap[0], [0, 4], rbap.ap[1]])
                    for c4 in range(k16 // 4):
                        Pn = pout.tile([128, 4, qb], F32, tag="pn")
                        nc.vector.tensor_mul(out=Pn,
                                             in0=Pt[:, c4 * 4:(c4 + 1) * 4, :],
                                             in1=rb4)
                        nc.sync.dma_start(
                            out=att_r[h, :, c4 * 4:(c4 + 1) * 4, qs], in_=Pn)

        # ---------- Phase 3: output projection (fp16) ----------
        with tc.tile_pool(name="ph3ps", bufs=2, space="PSUM") as wp, \
             tc.tile_pool(name="ph3y", bufs=2) as yp:
            for qc in range(nq):
                for m in range(4):
                    ps = wp.tile([128, qb], F32, tag="y")
                    for c in range(4):
                        nc.tensor.matmul(ps, lhsT=Wo16[:, c, m * 128:(m + 1) * 128],
                                         rhs=OTb[:, c, qc * qb:(qc + 1) * qb],
                                         start=(c == 0), stop=(c == 3))
                    y = yp.tile([128, qb], F32, tag="ysb")
                    nc.scalar.activation(out=y, in_=ps, func=AF.Identity,
                                         bias=bo_s[:, m:m + 1], scale=1.0)
                    nc.sync.dma_start(out=outT_d[m * 128:(m + 1) * 128,
                                                 qc * qb:(qc + 1) * qb], in_=y)

    nc.finalize()
    return nc


_NC_CACHE = {}


def _get_nc():
    if "nc" not in _NC_CACHE:
        _NC_CACHE["nc"] = build_nc()
    return _NC_CACHE["nc"]


def make_in_maps(query, key, value, attn_mask, gene_regulatory_matrix,
                 Wq, bq, Wk, bk, Wv, bv, Wo, bo):
    h = lambda a: np.asarray(a, np.float16)
    f = lambda a: np.asarray(a, np.float32)
    query, key, value = h(query), h(key), h(value)
    gene = h(gene_regulatory_matrix)
    mask = np.asarray(attn_mask).astype(np.int8)
    shared = {"Wq": h(Wq), "Wk": h(Wk), "Wv": h(Wv), "Wo": h(Wo),
              "bq": f(bq), "bk": f(bk), "bv": f(bv), "bo": f(bo)}
    in_maps = []
    for c in range(NCORES):
        b, half = c // 2, c % 2
        qs = slice(half * LQ, (half + 1) * LQ)
        in_maps.append({
            "qT": np.ascontiguousarray(query[b, qs].T),
            "kT": np.ascontiguousarray(key[b].T),
            "vT": np.ascontiguousarray(value[b].T),
            "maskT": np.ascontiguousarray(mask[b, qs].T),
            "geneT": np.ascontiguousarray(gene[qs].T),
            **shared,
        })
    return in_maps


def assemble(results):
    out = np.empty((B, L, E), np.float32)
    aw = np.empty((B, H, L, L), np.float32)
    for c in range(NCORES):
        b, half = c // 2, c % 2
        qs = slice(half * LQ, (half + 1) * LQ)
        aw[b, :, qs, :] = results[c]["attnwT"].transpose(0, 2, 1)
        out[b, qs, :] = results[c]["outT"].T
    return out, aw


def kernel(query, key, value, attn_mask, gene_regulatory_matrix,
           Wq, bq, Wk, bk, Wv, bv, Wo, bo):
    nc = _get_nc()
    in_maps = make_in_maps(query, key, value, attn_mask,
                           gene_regulatory_matrix,
                           Wq, bq, Wk, bk, Wv, bv, Wo, bo)
    res = run_bass_kernel_spmd(nc, in_maps, list(range(NCORES))).results
    return assemble(res)


# revision 26
# speedup vs baseline: 2.4796x; 1.0361x over previous
"""Trainium2 Bass kernel for MultiHeadAttention with gene-regulatory bias.

Reference computation (per batch b):
    q,k,v = (x @ W + b) split into 8 heads of 64
    scores = q k^T / 8 + gene[None]        (gene shared across batch/heads)
    scores = where(mask==0, -inf, scores)
    P = softmax(scores, axis=-1)           -> returned as attention_weights
    out = (P v) concat-heads @ Wo + bo

Sharding: 8 cores = (batch b, query-half). Each core handles 1024 query rows
x full 2048 keys x all 8 heads. No collectives; host reassembles.

Device dataflow (per core) is "transposed-score" oriented: S^T[k,q] tiles are
computed natively on the PE (so the P@V contraction needs no on-chip transpose
of the big score tensor), the softmax denominator is obtained by appending a
ones-column to V (one extra PSUM row in the P@V matmul), and normalization
happens along the free axis on the vector engine. The [k,q] -> [q,k] layout
fix of the returned weights is done on the host (pure layout/unshard step).

Numerics: fp32 matmuls run in the PE's slow LOW_HIGH mode, so the whole
pipeline is fp16 (x/W shards are cast to fp16 on the host; the 0/1 mask is
int8). fp16 keeps ~5e-4 relative accuracy per tensor and ~4e-3 end to end,
comfortably inside the absmax gate. exp is shifted by -4 (softmax-invariant
range guard); all matmul accumulation is fp32 PSUM; softmax normalization and
both outputs stay fp32.
"""

import numpy as np
from contextlib import ExitStack

import concourse.bass as bass
import concourse.bacc as bacc
import concourse.tile as tile
from concourse import mybir
from concourse.masks import make_identity
from concourse.bass_utils import run_bass_kernel_spmd

F32 = mybir.dt.float32
F16 = mybir.dt.float16
I8 = mybir.dt.int8
AF = mybir.ActivationFunctionType

B, L, E = 4, 2048, 512
H, DH = 8, 64
NCORES = 8
LQ = L // 2          # query rows per core
LK = L
NEG = -10000.0       # "minus infinity" for masked scores
ESHIFT = -4.0        # exp(s + ESHIFT): softmax-invariant fp16 range guard


def build_nc(lq=LQ, lk=LK):
    """Build the per-core Bass program (SPMD: same program, per-core data)."""
    nc = bacc.Bacc()
    qb = min(512, lq)            # query block for matmul moving operand
    nq = lq // qb                # query blocks
    k16 = lk // 128              # 128-row key chunks

    # ---- DRAM I/O (x/W pre-cast to fp16 on host; mask int8) ----
    qT_d = nc.dram_tensor("qT", [E, lq], F16, kind="ExternalInput")
    kT_d = nc.dram_tensor("kT", [E, lk], F16, kind="ExternalInput")
    vT_d = nc.dram_tensor("vT", [E, lk], F16, kind="ExternalInput")
    maskT_d = nc.dram_tensor("maskT", [lk, lq], I8, kind="ExternalInput")
    geneT_d = nc.dram_tensor("geneT", [lk, lq], F16, kind="ExternalInput")
    Wq_d = nc.dram_tensor("Wq", [E, E], F16, kind="ExternalInput")
    Wk_d = nc.dram_tensor("Wk", [E, E], F16, kind="ExternalInput")
    Wv_d = nc.dram_tensor("Wv", [E, E], F16, kind="ExternalInput")
    Wo_d = nc.dram_tensor("Wo", [E, E], F16, kind="ExternalInput")
    bq_d = nc.dram_tensor("bq", [E], F32, kind="ExternalInput")
    bk_d = nc.dram_tensor("bk", [E], F32, kind="ExternalInput")
    bv_d = nc.dram_tensor("bv", [E], F32, kind="ExternalInput")
    bo_d = nc.dram_tensor("bo", [E], F32, kind="ExternalInput")
    attnwT_d = nc.dram_tensor("attnwT", [H, lk, lq], F32, kind="ExternalOutput")
    outT_d = nc.dram_tensor("outT", [E, lq], F32, kind="ExternalOutput")

    # DRAM view with 128-partition layout
    att_r = attnwT_d.rearrange("h (n p) q -> h p n q", p=128)  # [H,128,k16,lq]

    with tile.TileContext(nc) as tc, ExitStack() as ctx:
        const = ctx.enter_context(tc.tile_pool(name="const", bufs=1))
        persist = ctx.enter_context(tc.tile_pool(name="persist", bufs=1))

        ident16 = const.tile([128, 128], F16, tag="id16")
        make_identity(nc, ident16)
        ones16 = const.tile([1, 128], F16, tag="ones")
        nc.vector.memset(ones16, 1.0)
        bv_row = const.tile([1, E], F16, tag="bvrow")
        nc.gpsimd.dma_start(out=bv_row, in_=bv_d.rearrange("(o e) -> o e", o=1))
        bq_s = const.tile([128, 4], F32, tag="bq")
        nc.sync.dma_start(out=bq_s, in_=bq_d.rearrange("(c p) -> p c", p=128))
        nc.scalar.mul(out=bq_s, in_=bq_s, mul=0.125)  # fold 1/sqrt(dh) into q
        bk_s = const.tile([128, 4], F32, tag="bk")
        nc.sync.dma_start(out=bk_s, in_=bk_d.rearrange("(c p) -> p c", p=128))
        bo_s = const.tile([128, 4], F32, tag="bo")
        nc.sync.dma_start(out=bo_s, in_=bo_d.rearrange("(c p) -> p c", p=128))
        eshift = const.tile([128, 1], F32, tag="eshift")
        nc.vector.memset(eshift, ESHIFT)

        # per-chunk tiles (not one big tile) so Tile's dependency tracking
        # lets phase 2b start as soon as the chunks it reads are ready
        Qf = [persist.tile([128, lq], F16, tag=f"Qf{c}", name=f"Qf{c}")
              for c in range(4)]
        Kf = [persist.tile([128, lk], F16, tag=f"Kf{c}", name=f"Kf{c}")
              for c in range(4)]
        V2 = persist.tile([128, k16, H, DH + 1], F16, tag="V2")  # [V_h | 1]
        SBT = [persist.tile([128, lq], F16, tag=f"SBT{kc}", name=f"SBT{kc}")
               for kc in range(k16)]
        OTb = persist.tile([128, 4, lq], F16, tag="OTb")   # attn out^T [E,lq]
        Wo16 = persist.tile([128, 4, E], F16, tag="Wo16")
        nc.sync.dma_start(out=Wo16, in_=Wo_d.rearrange("(c p) e -> p c e", p=128))

        nc.gpsimd.memset(V2[:, :, :, DH:DH + 1], 1.0)  # denominator ones-col

        # ---------- Phase 1: projections (all fp16 operands) ----------
        with tc.tile_pool(name="ph1x", bufs=2) as xpool, \
             tc.tile_pool(name="ph1w", bufs=2) as wpool, \
             tc.tile_pool(name="ph1ps", bufs=4, space="PSUM") as pp:

            def load_xw(x_dram, w_dram, xlen):
                xT = xpool.tile([128, 4, xlen], F16, tag="x")
                nc.sync.dma_start(out=xT, in_=x_dram.rearrange("(c p) n -> p c n", p=128))
                W = wpool.tile([128, 4, E], F16, tag="w")
                nc.sync.dma_start(out=W, in_=w_dram.rearrange("(c p) e -> p c e", p=128))
                return xT, W

            # Q^T = Wq^T x^T (scaled 1/8, bias bq/8)
            xT, W = load_xw(qT_d, Wq_d, lq)
            for m in range(4):
                for qc in range(nq):
                    sl = slice(qc * qb, (qc + 1) * qb)
                    ps = pp.tile([128, qb], F32, tag="ps")
                    for c in range(4):
                        nc.tensor.matmul(ps, lhsT=W[:, c, m * 128:(m + 1) * 128],
                                         rhs=xT[:, c, sl],
                                         start=(c == 0), stop=(c == 3))
                    nc.scalar.activation(out=Qf[m][:, sl], in_=ps, func=AF.Identity,
                                         bias=bq_s[:, m:m + 1], scale=0.125)
            # K^T = Wk^T x^T
            xT, W = load_xw(kT_d, Wk_d, lk)
            for m in range(4):
                for qc in range(lk // 512):
                    sl = slice(qc * 512, (qc + 1) * 512)
                    ps = pp.tile([128, 512], F32, tag="ps")
                    for c in range(4):
                        nc.tensor.matmul(ps, lhsT=W[:, c, m * 128:(m + 1) * 128],
                                         rhs=xT[:, c, sl],
                                         start=(c == 0), stop=(c == 3))
                    nc.scalar.activation(out=Kf[m][:, sl], in_=ps, func=AF.Identity,
                                         bias=bk_s[:, m:m + 1], scale=1.0)
            # V = value Wv + bv (natural [k, dh] layout, per-head + ones col)
            xT, W = load_xw(vT_d, Wv_d, lk)
            for kc in range(k16):
                ps = pp.tile([128, E], F32, tag="psv")
                for c in range(4):
                    nc.tensor.matmul(ps, lhsT=xT[:, c, kc * 128:(kc + 1) * 128],
                                     rhs=W[:, c, :], start=(c == 0), stop=False)
                nc.tensor.matmul(ps, lhsT=ones16, rhs=bv_row,
                                 start=False, stop=True)  # += bv broadcast
                nc.scalar.activation(
                    out=V2[:, kc, :, 0:DH],
                    in_=ps.rearrange("p (h d) -> p h d", h=H),
                    func=AF.Copy)

        # ---------- Phase 2a: masked gene bias (transposed), fp16 ----------
        with tc.tile_pool(name="ph2a", bufs=2) as mg:
            for kc in range(k16):
                mt = mg.tile([128, lq], I8, tag="m")
                nc.sync.dma_start(out=mt, in_=maskT_d[kc * 128:(kc + 1) * 128, :])
                gt = mg.tile([128, lq], F16, tag="g")
                nc.sync.dma_start(out=gt, in_=geneT_d[kc * 128:(kc + 1) * 128, :])
                nc.gpsimd.memset(SBT[kc][:, :], NEG)
                nc.vector.copy_predicated(out=SBT[kc][:, :], mask=mt, data=gt)

        # ---------- Phase 2b: attention ----------
        with tc.tile_pool(name="spsum", bufs=2, space="PSUM") as spsum, \
             tc.tile_pool(name="opsum", bufs=2, space="PSUM") as opsum, \
             tc.tile_pool(name="strip", bufs=3) as strip, \
             tc.tile_pool(name="pout", bufs=4) as pout, \
             tc.tile_pool(name="small", bufs=4) as small, \
             tc.tile_pool(name="ph3ps", bufs=2, space="PSUM") as wp, \
             tc.tile_pool(name="ph3y", bufs=2) as yp:
            for qg in range(nq):
                for h in range(H):
                    hp, hc = (h % 2) * 64, h // 2
                    qs = slice(qg * qb, (qg + 1) * qb)
                    Pt = strip.tile([128, k16, qb], F16, tag="pt")
                    for kt in range(k16 // 2):
                        ps = spsum.tile([128, 2, qb], F32, tag="s")
                        for j in range(2):
                            kc = 2 * kt + j
                            ks = slice(kc * 128, (kc + 1) * 128)
                            # S^T[k,q] = K_h Q_h^T  (q pre-scaled by 1/8)
                            nc.tensor.matmul(ps[:, j, :],
                                             lhsT=Kf[hc][hp:hp + 64, ks],
                                             rhs=Qf[hc][hp:hp + 64, qs],
                                             start=True, stop=False)
                            # += gene/mask bias via identity weights
                            nc.tensor.matmul(ps[:, j, :], lhsT=ident16,
                                             rhs=SBT[kc][:, qs],
                                             start=False, stop=True)
                        nc.scalar.activation(out=Pt[:, 2 * kt:2 * kt + 2, :],
                                             in_=ps, func=AF.Exp,
                                             bias=eshift, scale=1.0)
                    # O'^T[dh,q] (+ denominator row) = [V_h|1]^T P~^T
                    ops = opsum.tile([DH + 1, qb], F32, tag="o")
                    for kc in range(k16):
                        nc.tensor.matmul(ops, lhsT=V2[:, kc, h, :], rhs=Pt[:, kc, :],
                                         start=(kc == 0), stop=(kc == k16 - 1))
                    rd = small.tile([1, qb], F32, tag="rd")
                    nc.vector.reciprocal(out=rd, in_=ops[DH:DH + 1, :])
                    # broadcast 1/denom across partitions on GpSimd
                    # (keeps the PE stream free of the reciprocal dependency)
                    rb = small.tile([128, qb], F32, tag="rbs")
                    nc.gpsimd.partition_broadcast(rb, rd)
                    # normalized attention out^T rows for this head (fp16)
                    nc.vector.tensor_mul(out=OTb[hp:hp + 64, hc, qs],
                                         in0=ops[0:DH, :], in1=rb[0:DH, :])
                    # normalized weights -> DRAM f32, 4 key-chunks per DMA
                    rbap = rb[:, :]
                    rb4 = bass.AP(tensor=rbap.tensor, offset=rbap.offset,
                                  ap=[rbap.ap[0], [0, 4], rbap.ap[1]])
                    for c4 in range(k16 // 4):
                        Pn = pout.tile([128, 4, qb], F32, tag="pn")
                        nc.vector.tensor_mul(out=Pn,
                                             in0=Pt[:, c4 * 4:(c4 + 1) * 4, :],
                                             in1=rb4)
                        nc.sync.dma_start(
                            out=att_r[h, :, c4 * 4:(c4 + 1) * 4, qs], in_=Pn)

                # output projection for this query group (overlaps next group)
                for m in range(4):
                    ps = wp.tile([128, qb], F32, tag="y")
                    for c in range(4):
                        nc.tensor.matmul(ps, lhsT=Wo16[:, c, m * 128:(m + 1) * 128],
                                         rhs=OTb[:, c, qs],
                                         start=(c == 0), stop=(c == 3))
                    y = yp.tile([128, qb], F32, tag="ysb")
                    nc.scalar.activation(out=y, in_=ps, func=AF.Identity,
                                         bias=bo_s[:, m:m + 1], scale=1.0)
                    nc.sync.dma_start(out=outT_d[m * 128:(m + 1) * 128, qs], in_=y)

    nc.finalize()
    return nc


_NC_CACHE = {}


def _get_nc():
    if "nc" not in _NC_CACHE:
        _NC_CACHE["nc"] = build_nc()
    return _NC_CACHE["nc"]


def make_in_maps(query, key, value, attn_mask, gene_regulatory_matrix,
                 Wq, bq, Wk, bk, Wv, bv, Wo, bo):
    h = lambda a: np.asarray(a, np.float16)
    f = lambda a: np.asarray(a, np.float32)
    query, key, value = h(query), h(key), h(value)
    gene = h(gene_regulatory_matrix)
    mask = np.asarray(attn_mask).astype(np.int8)
    shared = {"Wq": h(Wq), "Wk": h(Wk), "Wv": h(Wv), "Wo": h(Wo),
              "bq": f(bq), "bk": f(bk), "bv": f(bv), "bo": f(bo)}
    in_maps = []
    for c in range(NCORES):
        b, half = c // 2, c % 2
        qs = slice(half * LQ, (half + 1) * LQ)
        in_maps.append({
            "qT": np.ascontiguousarray(query[b, qs].T),
            "kT": np.ascontiguousarray(key[b].T),
            "vT": np.ascontiguousarray(value[b].T),
            "maskT": np.ascontiguousarray(mask[b, qs].T),
            "geneT": np.ascontiguousarray(gene[qs].T),
            **shared,
        })
    return in_maps


def assemble(results):
    out = np.empty((B, L, E), np.float32)
    aw = np.empty((B, H, L, L), np.float32)
    for c in range(NCORES):
        b, half = c // 2, c % 2
        qs = slice(half * LQ, (half + 1) * LQ)
        aw[b, :, qs, :] = results[c]["attnwT"].transpose(0, 2, 1)
        out[b, qs, :] = results[c]["outT"].T
    return out, aw


def kernel(query, key, value, attn_mask, gene_regulatory_matrix,
           Wq, bq, Wk, bk, Wv, bv, Wo, bo):
    nc = _get_nc()
    in_maps = make_in_maps(query, key, value, attn_mask,
                           gene_regulatory_matrix,
                           Wq, bq, Wk, bk, Wv, bv, Wo, bo)
    res = run_bass_kernel_spmd(nc, in_maps, list(range(NCORES))).results
    return assemble(res)


# revision 28
# speedup vs baseline: 2.5644x; 1.0342x over previous
"""Trainium2 Bass kernel for MultiHeadAttention with gene-regulatory bias.

Reference computation (per batch b):
    q,k,v = (x @ W + b) split into 8 heads of 64
    scores = q k^T / 8 + gene[None]        (gene shared across batch/heads)
    scores = where(mask==0, -inf, scores)
    P = softmax(scores, axis=-1)           -> returned as attention_weights
    out = (P v) concat-heads @ Wo + bo

Sharding: 8 cores = (batch b, query-half). Each core handles 1024 query rows
x full 2048 keys x all 8 heads. No collectives; host reassembles.

Device dataflow (per core) is "transposed-score" oriented: S^T[k,q] tiles are
computed natively on the PE (so the P@V contraction needs no on-chip transpose
of the big score tensor), the softmax denominator is obtained by appending a
ones-column to V (one extra PSUM row in the P@V matmul), and normalization
happens along the free axis on the vector engine. The [k,q] -> [q,k] layout
fix of the returned weights is done on the host (pure layout/unshard step).

Numerics: fp32 matmuls run in the PE's slow LOW_HIGH mode, so the whole
pipeline is fp16 (x/W shards are cast to fp16 on the host; the 0/1 mask is
int8). fp16 keeps ~5e-4 relative accuracy per tensor and ~4e-3 end to end,
comfortably inside the absmax gate. exp is shifted by -4 (softmax-invariant
range guard); all matmul accumulation is fp32 PSUM; softmax normalization and
both outputs stay fp32.
"""

import numpy as np
from contextlib import ExitStack

import concourse.bass as bass
import concourse.bacc as bacc
import concourse.tile as tile
from concourse import mybir
from concourse.masks import make_identity
from concourse.bass_utils import run_bass_kernel_spmd

F32 = mybir.dt.float32
F16 = mybir.dt.float16
I8 = mybir.dt.int8
AF = mybir.ActivationFunctionType

B, L, E = 4, 2048, 512
H, DH = 8, 64
NCORES = 8
LQ = L // 2          # query rows per core
LK = L
NEG = -10000.0       # "minus infinity" for masked scores
ESHIFT = -4.0        # exp(s + ESHIFT): softmax-invariant fp16 range guard


def build_nc(lq=LQ, lk=LK):
    """Build the per-core Bass program (SPMD: same program, per-core data)."""
    nc = bacc.Bacc()
    qb = min(512, lq)            # query block for matmul moving operand
    nq = lq // qb                # query blocks
    k16 = lk // 128              # 128-row key chunks

    # ---- DRAM I/O (x/W pre-cast to fp16 on host; mask int8) ----
    qT_d = nc.dram_tensor("qT", [E, lq], F16, kind="ExternalInput")
    kT_d = nc.dram_tensor("kT", [E, lk], F16, kind="ExternalInput")
    vT_d = nc.dram_tensor("vT", [E, lk], F16, kind="ExternalInput")
    maskT_d = nc.dram_tensor("maskT", [lk, lq], I8, kind="ExternalInput")
    geneT_d = nc.dram_tensor("geneT", [lk, lq], F16, kind="ExternalInput")
    Wq_d = nc.dram_tensor("Wq", [E, E], F16, kind="ExternalInput")
    Wk_d = nc.dram_tensor("Wk", [E, E], F16, kind="ExternalInput")
    Wv_d = nc.dram_tensor("Wv", [E, E], F16, kind="ExternalInput")
    Wo_d = nc.dram_tensor("Wo", [E, E], F16, kind="ExternalInput")
    bq_d = nc.dram_tensor("bq", [E], F32, kind="ExternalInput")
    bk_d = nc.dram_tensor("bk", [E], F32, kind="ExternalInput")
    bv_d = nc.dram_tensor("bv", [E], F32, kind="ExternalInput")
    bo_d = nc.dram_tensor("bo", [E], F32, kind="ExternalInput")
    attnwT_d = nc.dram_tensor("attnwT", [H, lk, lq], F32, kind="ExternalOutput")
    outT_d = nc.dram_tensor("outT", [E, lq], F32, kind="ExternalOutput")

    # DRAM view with 128-partition layout
    att_r = attnwT_d.rearrange("h (n p) q -> h p n q", p=128)  # [H,128,k16,lq]

    with tile.TileContext(nc) as tc, ExitStack() as ctx:
        const = ctx.enter_context(tc.tile_pool(name="const", bufs=1))
        persist = ctx.enter_context(tc.tile_pool(name="persist", bufs=1))

        ident16 = const.tile([128, 128], F16, tag="id16")
        make_identity(nc, ident16)
        ones16 = const.tile([1, 128], F16, tag="ones")
        nc.vector.memset(ones16, 1.0)
        bv_row = const.tile([1, E], F16, tag="bvrow")
        nc.gpsimd.dma_start(out=bv_row, in_=bv_d.rearrange("(o e) -> o e", o=1))
        bq_s = const.tile([128, 4], F32, tag="bq")
        nc.sync.dma_start(out=bq_s, in_=bq_d.rearrange("(c p) -> p c", p=128))
        nc.scalar.mul(out=bq_s, in_=bq_s, mul=0.125)  # fold 1/sqrt(dh) into q
        bk_s = const.tile([128, 4], F32, tag="bk")
        nc.sync.dma_start(out=bk_s, in_=bk_d.rearrange("(c p) -> p c", p=128))
        bo_s = const.tile([128, 4], F32, tag="bo")
        nc.sync.dma_start(out=bo_s, in_=bo_d.rearrange("(c p) -> p c", p=128))
        eshift = const.tile([128, 1], F32, tag="eshift")
        nc.vector.memset(eshift, ESHIFT)

        # per-chunk tiles (not one big tile) so Tile's dependency tracking
        # lets phase 2b start as soon as the chunks it reads are ready
        Qf = [persist.tile([128, lq], F16, tag=f"Qf{c}", name=f"Qf{c}")
              for c in range(4)]
        Kf = [persist.tile([128, lk], F16, tag=f"Kf{c}", name=f"Kf{c}")
              for c in range(4)]
        V2 = persist.tile([128, k16, H, DH + 1], F16, tag="V2")  # [V_h | 1]
        SBT = [persist.tile([128, lq], F16, tag=f"SBT{kc}", name=f"SBT{kc}")
               for kc in range(k16)]
        OTb = persist.tile([128, 4, lq], F16, tag="OTb")   # attn out^T [E,lq]
        Wo16 = persist.tile([128, 4, E], F16, tag="Wo16")
        nc.sync.dma_start(out=Wo16, in_=Wo_d.rearrange("(c p) e -> p c e", p=128))

        nc.gpsimd.memset(V2[:, :, :, DH:DH + 1], 1.0)  # denominator ones-col

        # ---------- Phase 1: projections (all fp16 operands) ----------
        with tc.tile_pool(name="ph1x", bufs=2) as xpool, \
             tc.tile_pool(name="ph1w", bufs=2) as wpool, \
             tc.tile_pool(name="ph1ps", bufs=4, space="PSUM") as pp:

            def load_xw(x_dram, w_dram, xlen):
                xT = xpool.tile([128, 4, xlen], F16, tag="x")
                nc.sync.dma_start(out=xT, in_=x_dram.rearrange("(c p) n -> p c n", p=128))
                W = wpool.tile([128, 4, E], F16, tag="w")
                nc.sync.dma_start(out=W, in_=w_dram.rearrange("(c p) e -> p c e", p=128))
                return xT, W

            # Q^T = Wq^T x^T (scaled 1/8, bias bq/8)
            xT, W = load_xw(qT_d, Wq_d, lq)
            for m in range(4):
                for qc in range(nq):
                    sl = slice(qc * qb, (qc + 1) * qb)
                    ps = pp.tile([128, qb], F32, tag="ps")
                    for c in range(4):
                        nc.tensor.matmul(ps, lhsT=W[:, c, m * 128:(m + 1) * 128],
                                         rhs=xT[:, c, sl],
                                         start=(c == 0), stop=(c == 3))
                    nc.scalar.activation(out=Qf[m][:, sl], in_=ps, func=AF.Identity,
                                         bias=bq_s[:, m:m + 1], scale=0.125)
            # K^T = Wk^T x^T
            xT, W = load_xw(kT_d, Wk_d, lk)
            for m in range(4):
                for qc in range(lk // 512):
                    sl = slice(qc * 512, (qc + 1) * 512)
                    ps = pp.tile([128, 512], F32, tag="ps")
                    for c in range(4):
                        nc.tensor.matmul(ps, lhsT=W[:, c, m * 128:(m + 1) * 128],
                                         rhs=xT[:, c, sl],
                                         start=(c == 0), stop=(c == 3))
                    nc.scalar.activation(out=Kf[m][:, sl], in_=ps, func=AF.Identity,
                                         bias=bk_s[:, m:m + 1], scale=1.0)
            # V = value Wv + bv (natural [k, dh] layout, per-head + ones col)
            xT, W = load_xw(vT_d, Wv_d, lk)
            for kc in range(k16):
                ps = pp.tile([128, E], F32, tag="psv")
                for c in range(4):
                    nc.tensor.matmul(ps, lhsT=xT[:, c, kc * 128:(kc + 1) * 128],
                                     rhs=W[:, c, :], start=(c == 0), stop=False)
                nc.tensor.matmul(ps, lhsT=ones16, rhs=bv_row,
                                 start=False, stop=True)  # += bv broadcast
                nc.scalar.activation(
                    out=V2[:, kc, :, 0:DH],
                    in_=ps.rearrange("p (h d) -> p h d", h=H),
                    func=AF.Copy)

        # ---------- Phase 2a: masked gene bias (transposed), fp16 ----------
        with tc.tile_pool(name="ph2a", bufs=4) as mg:
            for kc in range(k16):
                # issue on the (idle) gpsimd queue so these transfers overlap
                # the projection phase instead of queueing behind its loads
                mt = mg.tile([128, lq], I8, tag="m")
                nc.gpsimd.dma_start(out=mt, in_=maskT_d[kc * 128:(kc + 1) * 128, :])
                gt = mg.tile([128, lq], F16, tag="g")
                nc.gpsimd.dma_start(out=gt, in_=geneT_d[kc * 128:(kc + 1) * 128, :])
                nc.gpsimd.memset(SBT[kc][:, :], NEG)
                nc.vector.copy_predicated(out=SBT[kc][:, :], mask=mt, data=gt)

        # ---------- Phase 2b: attention ----------
        with tc.tile_pool(name="spsum", bufs=2, space="PSUM") as spsum, \
             tc.tile_pool(name="opsum", bufs=2, space="PSUM") as opsum, \
             tc.tile_pool(name="strip", bufs=3) as strip, \
             tc.tile_pool(name="pout", bufs=4) as pout, \
             tc.tile_pool(name="small", bufs=4) as small, \
             tc.tile_pool(name="ph3ps", bufs=2, space="PSUM") as wp, \
             tc.tile_pool(name="ph3y", bufs=2) as yp:
            for qg in range(nq):
                for h in range(H):
                    hp, hc = (h % 2) * 64, h // 2
                    qs = slice(qg * qb, (qg + 1) * qb)
                    Pt = strip.tile([128, k16, qb], F16, tag="pt")
                    for kt in range(k16 // 2):
                        ps = spsum.tile([128, 2, qb], F32, tag="s")
                        for j in range(2):
                            kc = 2 * kt + j
                            ks = slice(kc * 128, (kc + 1) * 128)
                            # S^T[k,q] = K_h Q_h^T  (q pre-scaled by 1/8)
                            nc.tensor.matmul(ps[:, j, :],
                                             lhsT=Kf[hc][hp:hp + 64, ks],
                                             rhs=Qf[hc][hp:hp + 64, qs],
                                             start=True, stop=False)
                            # += gene/mask bias via identity weights
                            nc.tensor.matmul(ps[:, j, :], lhsT=ident16,
                                             rhs=SBT[kc][:, qs],
                                             start=False, stop=True)
                        nc.scalar.activation(out=Pt[:, 2 * kt:2 * kt + 2, :],
                                             in_=ps, func=AF.Exp,
                                             bias=eshift, scale=1.0)
                    # O'^T[dh,q] (+ denominator row) = [V_h|1]^T P~^T
                    ops = opsum.tile([DH + 1, qb], F32, tag="o")
                    for kc in range(k16):
                        nc.tensor.matmul(ops, lhsT=V2[:, kc, h, :], rhs=Pt[:, kc, :],
                                         start=(kc == 0), stop=(kc == k16 - 1))
                    rd = small.tile([1, qb], F32, tag="rd")
                    nc.vector.reciprocal(out=rd, in_=ops[DH:DH + 1, :])
                    # broadcast 1/denom across partitions on GpSimd
                    # (keeps the PE stream free of the reciprocal dependency)
                    rb = small.tile([128, qb], F32, tag="rbs")
                    nc.gpsimd.partition_broadcast(rb, rd)
                    # normalized attention out^T rows for this head (fp16)
                    nc.vector.tensor_mul(out=OTb[hp:hp + 64, hc, qs],
                                         in0=ops[0:DH, :], in1=rb[0:DH, :])
                    # normalized weights -> DRAM f32, 4 key-chunks per DMA
                    rbap = rb[:, :]
                    rb4 = bass.AP(tensor=rbap.tensor, offset=rbap.offset,
                                  ap=[rbap.ap[0], [0, 4], rbap.ap[1]])
                    for c4 in range(k16 // 4):
                        Pn = pout.tile([128, 4, qb], F32, tag="pn")
                        nc.vector.tensor_mul(out=Pn,
                                             in0=Pt[:, c4 * 4:(c4 + 1) * 4, :],
                                             in1=rb4)
                        nc.sync.dma_start(
                            out=att_r[h, :, c4 * 4:(c4 + 1) * 4, qs], in_=Pn)

                # output projection for this query group (overlaps next group)
                for m in range(4):
                    ps = wp.tile([128, qb], F32, tag="y")
                    for c in range(4):
                        nc.tensor.matmul(ps, lhsT=Wo16[:, c, m * 128:(m + 1) * 128],
                                         rhs=OTb[:, c, qs],
                                         start=(c == 0), stop=(c == 3))
                    y = yp.tile([128, qb], F32, tag="ysb")
                    nc.scalar.activation(out=y, in_=ps, func=AF.Identity,
                                         bias=bo_s[:, m:m + 1], scale=1.0)
                    nc.sync.dma_start(out=outT_d[m * 128:(m + 1) * 128, qs], in_=y)

    nc.finalize()
    return nc


_NC_CACHE = {}


def _get_nc():
    if "nc" not in _NC_CACHE:
        _NC_CACHE["nc"] = build_nc()
    return _NC_CACHE["nc"]


def make_in_maps(query, key, value, attn_mask, gene_regulatory_matrix,
                 Wq, bq, Wk, bk, Wv, bv, Wo, bo):
    h = lambda a: np.asarray(a, np.float16)
    f = lambda a: np.asarray(a, np.float32)
    query, key, value = h(query), h(key), h(value)
    gene = h(gene_regulatory_matrix)
    mask = np.asarray(attn_mask).astype(np.int8)
    shared = {"Wq": h(Wq), "Wk": h(Wk), "Wv": h(Wv), "Wo": h(Wo),
              "bq": f(bq), "bk": f(bk), "bv": f(bv), "bo": f(bo)}
    in_maps = []
    for c in range(NCORES):
        b, half = c // 2, c % 2
        qs = slice(half * LQ, (half + 1) * LQ)
        in_maps.append({
            "qT": np.ascontiguousarray(query[b, qs].T),
            "kT": np.ascontiguousarray(key[b].T),
            "vT": np.ascontiguousarray(value[b].T),
            "maskT": np.ascontiguousarray(mask[b, qs].T),
            "geneT": np.ascontiguousarray(gene[qs].T),
            **shared,
        })
    return in_maps


def assemble(results):
    out = np.empty((B, L, E), np.float32)
    aw = np.empty((B, H, L, L), np.float32)
    for c in range(NCORES):
        b, half = c // 2, c % 2
        qs = slice(half * LQ, (half + 1) * LQ)
        aw[b, :, qs, :] = results[c]["attnwT"].transpose(0, 2, 1)
        out[b, qs, :] = results[c]["outT"].T
    return out, aw


def kernel(query, key, value, attn_mask, gene_regulatory_matrix,
           Wq, bq, Wk, bk, Wv, bv, Wo, bo):
    nc = _get_nc()
    in_maps = make_in_maps(query, key, value, attn_mask,
                           gene_regulatory_matrix,
                           Wq, bq, Wk, bk, Wv, bv, Wo, bo)
    res = run_bass_kernel_spmd(nc, in_maps, list(range(NCORES))).results
    return assemble(res)
